# revision 96
# baseline (speedup 1.0000x reference)
"""Distributed sparse-MoE routing kernel for 8 Trainium2 NeuronCores (v4).

Algorithm notes
---------------
The reference routes T=16384 tokens (top-1 of E=8 experts, capacity C=100,
tokens past capacity dropped) and applies ONE shared expert weight (H -> H
Linear).  Because the expert weight is shared, the output collapses to

    out[t] = gate_t * (x_t @ W + b)   if token t wins a capacity slot
           = 0                        otherwise

so only <= E*C = 800 of 16384 tokens need the big matmul.  Token t (choosing
expert e) wins a slot iff fewer than C earlier tokens (global order) chose e.

Distribution: tokens sharded over 8 cores in 32-token blocks, strided (core k
owns blocks b with b % 8 == k).  Each core computes per-block expert counts
locally; one small AllGather (64x8 per core) shares them; small matmuls turn
the gathered table into per-block capacity bases.  Everything else is local.

v2-v4 changes vs the 125-138us v1 (trace-driven):
- Router runs as an fp16 hi/lo split instead of fp32: x = x_h + x_l and
  w_gate = w_h + w_l (fp16 pairs, exact decomposition).  Stationary is
  [w_h | w_l] packed [128,16]; streaming x_h then x_l accumulates all four
  cross terms in one PSUM, one vector add folds the halves.  fp16 streams at
  1 cycle/row vs fp32's 4, so the router drops ~30us -> ~8us of PE time.
  Measured offline on the seed-0 data: max logit error 2.9e-6 vs min top-2
  gap 1.39e-5 (16x margin), zero argmax flips.
- Softmax/masks batched over all 16 token tiles in one pass (was 4 groups).
- Post-AG path cut 45us -> ~28us: addbase repacked on the PE (Dexp expand +
  Q64 matmul) instead of a 4us DRAM round-trip; prefix scan replaced with
  LT16/diag matmuls; masks/keep/kf/M/tsv/cmpT all fp16 (small exact
  integers; fp32 matmuls get split into 2 half-speed passes by the
  backend, fp16 does not); gather reads an fp16 copy of x (half the
  indirect-DMA bytes) and transposes feed an fp16 expert matmul (w_expert
  fp16: ~3.5e-4 rel output error, tolerance is 2e-2).
- Gate scale folded into the expert-output PSUM->SBUF copy.
- Scatter pads route to a dummy DRAM row (out has 2049 rows; pad slots
  accumulate idx 0 and add 2048): no bounds-check, trigger 3.6us -> 1.1us.
- Output split into two [2049, 512] tensors so each column half scatters
  as soon as its expert matmul half finishes.
- Constants consolidated into blob DMAs (hot f32 / cold f32 / f16) instead
  of ~12 tiny DMAs at ~0.6us of sync-engine trigger time each.

Measured constraints on this fleet (do not re-derive):
- The first collective is floor-pinned: a runtime CC barrier starts at a
  CONSTANT ~21.4us (independent of kernel content), runs 28-45us (= launch
  skew, varies run to run), then AG start = barrier_end + ~11us fixed, AG
  mesh 11-15us for 2KB.  Triggering the AG earlier does NOT move it: v2's
  trigger at ~50 vs v1's ~57 gave the same AG completion.  exec_time ~=
  21.4 + skew + ~24 + post-AG(27.6) + drain, so run-to-run spread is
  +-12us from skew alone.  Do NOT burn effort below the trigger floor.
- Pre-AG is DMA-bound, not PE-bound: 8MB of x halves at ~250GB/s effective
  lands ~44us; the fp16 router rides along at zero marginal cost.
- The PE duty-cycles to 50% (HAM k=4/n=8) after ~30us of activity.
- DMA: 16 HW queues, ~22GB/s each when all contend (358GB/s/core cap);
  each dma_start costs ~0.61us of sync-engine trigger time.
- indirect DMA: offset APs / outputs must start at partition 0 (a split
  gather with base-64 slices crashes the device with
  NRT_EXEC_UNIT_UNRECOVERABLE); the indirect-side AP must be a full
  tensor with offset 0 (hence the two separate out0/out1 tensors).
- bitcast-to-F32R of a DMA-produced tile is rejected by the bir verifier
  ("not rounded to FP32r").
- exec_time is core 0's span (only core 0 is profiled by default).
- ~10% of fresh runs crash with NRT_EXEC_UNIT_UNRECOVERABLE; an in-process
  retry only recovers if the jax/PJRT backend is torn down first (see
  kernel()).
- End-of-kernel drain (TileContext teardown semaphore handshakes) is ~8us.
"""
import os
import sys
import types
from contextlib import ExitStack

sys.path.insert(0, "/opt/trn_rl_repo")

import numpy as np

import concourse.bass as bass
import concourse.bacc as bacc
import concourse.mybir as mybir
import concourse.tile as tile
from concourse import bass_utils

F32 = mybir.dt.float32
F32R = mybir.dt.float32r
F16 = mybir.dt.float16
I32 = mybir.dt.int32
AX = mybir.AxisListType
ALU = mybir.AluOpType
ACT = mybir.ActivationFunctionType

P = 128          # SBUF partitions / tile rows
H = 1024         # hidden dim
E = 8            # experts
C = 100          # capacity
NCORES = 8
T_LOC = 2048     # tokens per core
NTILE = T_LOC // P   # 16 token tiles per core
NCH = H // P         # 8 hidden chunks
BLK = 32             # token block size for sharding
NBLK = T_LOC // BLK  # 64 local blocks per core
KMAX = 128           # max compacted (kept) tokens per core (<=114 actual)
TRASH = T_LOC        # dummy out row for pad slots

# cold consts blob column offsets
CC_TRI = 0
CC_IOTA = 128
CC_TIDX = 256
CC_MISC = 272        # rows 0-3 eexp (f32 copy; f16 mirror lives in cf16)
CC_ONESC = 400
CC_NEG1 = 401
CC_H64 = 409
CC_MASKK = 473
CC_ONES1 = 473 + 4 * NBLK    # 729, rows 0-15 all-ones [16, 128]
CC_LT16 = CC_ONES1 + 128     # 857, rows 0-15: LT16[k,m] = k<m
CC_DEXP = CC_LT16 + 16       # 873, rows 0-63: Dexp[j, 8i+e] = (j//4==i)
CC_W = CC_DEXP + 128         # 1001

# f16 consts blob column offsets
CF_EEXP = 0                  # rows 0-3
CF_Q64 = 128                 # rows 0-63
CF_IOTA = 132
CF_IDENT = 260
CF_TRI = 388
CF_ESUM = 516
CF_ONESC = 520
CF_W = 521


def build(has_bias=False):
    """Build + compile the SPMD program (identical on all 8 cores)."""
    nc = bacc.Bacc("TRN2", target_bir_lowering=False, debug=False,
                   num_devices=NCORES)

    x = nc.dram_tensor("x16", [T_LOC, H], F16, kind="ExternalInput")
    xth = nc.dram_tensor("xth", [H, T_LOC], F16, kind="ExternalInput")
    xtl = nc.dram_tensor("xtl", [H, T_LOC], F16, kind="ExternalInput")
    wgcat = nc.dram_tensor("wgcat", [H, 2 * E], F16, kind="ExternalInput")
    we = nc.dram_tensor("w_expert", [H, H], F16, kind="ExternalInput")
    be = (nc.dram_tensor("b_expert", [1, H], F16, kind="ExternalInput")
          if has_bias else None)
    ch = nc.dram_tensor("ch", [P, 132], F32, kind="ExternalInput")
    cc = nc.dram_tensor("cc", [P, CC_W], F32, kind="ExternalInput")
    cf = nc.dram_tensor("cf", [P, CF_W], F16, kind="ExternalInput")

    out0 = nc.dram_tensor("out0", [T_LOC + 1, H // 2], F32,
                          kind="ExternalOutput")
    out1 = nc.dram_tensor("out1", [T_LOC + 1, H // 2], F32,
                          kind="ExternalOutput")

    with tile.TileContext(nc) as tc:
        _body(nc, tc, x, xth, xtl, wgcat, we, be, ch, cc, cf, out0, out1)

    nc.compile()
    return nc


def _body(nc, tc, x, xth, xtl, wgcat, we, be, ch, cc, cf, out0, out1):
    with ExitStack() as top:
        sb = top.enter_context(tc.tile_pool(name="sb", bufs=1))
        st = top.enter_context(tc.tile_pool(name="st", bufs=4))
        dram = top.enter_context(tc.tile_pool(name="dram", bufs=1, space="DRAM"))

        # ---- DMA schedule -------------------------------------------------
        # Trigger order is program order on the sync engine (~0.61us each).
        # wgcat + hot consts first (router + transposes need them), then the
        # x halves interleaved by group so the router starts on group 0
        # early, then cold consts (first used in the AG stall), then w_expert
        # (first used in phase C).
        wg_sb = sb.tile([P, NCH * 2 * E], F16, tag="wgcat")
        nc.sync.dma_start(wg_sb[:].rearrange("p (c e) -> p c e", c=NCH),
                          wgcat[:, :].rearrange("(c p) e -> p c e", p=P))
        ch_sb = sb.tile([P, 132], F32, tag="ch")
        nc.sync.dma_start(ch_sb[:], ch[:, :])

        xh_sb = sb.tile([P, NCH * T_LOC], F16, tag="xh")
        xl_sb = sb.tile([P, NCH * T_LOC], F16, tag="xl")
        xh3 = xh_sb[:].rearrange("p (c t) -> p c t", c=NCH)
        xl3 = xl_sb[:].rearrange("p (c t) -> p c t", c=NCH)
        cold_done = False
        cc_sb = sb.tile([P, CC_W], F32, tag="cc")
        for g in range(4):
            for src, dst in ((xth, xh3), (xtl, xl3)):
                for q in range(2):
                    nc.sync.dma_start(
                        dst[:, 4 * q:4 * q + 4, g * 512:(g + 1) * 512],
                        src[4 * q * P:(4 * q + 4) * P,
                            g * 512:(g + 1) * 512].rearrange(
                                "(c p) t -> p c t", p=P))
            if g == 1 and not cold_done:
                nc.sync.dma_start(cc_sb[:], cc[:, :])
                cold_done = True
        cf_sb = sb.tile([P, CF_W], F16, tag="cf")
        nc.sync.dma_start(cf_sb[:], cf[:, :])

        we_sb = sb.tile([P, NCH * H], F16, tag="we")
        for c in range(NCH):
            nc.sync.dma_start(we_sb[:, c * H:(c + 1) * H],
                              we[c * P:(c + 1) * P, :])
        be_sb = None
        if be is not None:
            be_sb = sb.tile([1, H], F16, tag="be")
            nc.sync.dma_start(be_sb[:], be[:, :])

        # const views
        ident = ch_sb[:, 0:128]
        esum = ch_sb[:, 128:132]
        tri = cc_sb[:, CC_TRI:CC_TRI + 128]
        iota = cc_sb[:, CC_IOTA:CC_IOTA + 128]
        tidx = cc_sb[:, CC_TIDX:CC_TIDX + NTILE]
        ones1 = cc_sb[0:1, CC_ONES1:CC_ONES1 + 128]
        ones16 = cc_sb[0:16, CC_ONES1:CC_ONES1 + 128]
        lt16 = cc_sb[0:16, CC_LT16:CC_LT16 + 16]
        dexp = cc_sb[0:64, CC_DEXP:CC_DEXP + 128]
        onescol = cc_sb[:, CC_ONESC:CC_ONESC + 1]
        neg1 = cc_sb[0:1, CC_NEG1:CC_NEG1 + E]
        eexp16 = cf_sb[0:4, CF_EEXP:CF_EEXP + 128]
        q64 = cf_sb[0:64, CF_Q64:CF_Q64 + 4]
        iota16f = cf_sb[:, CF_IOTA:CF_IOTA + 128]
        identf = cf_sb[:, CF_IDENT:CF_IDENT + 128]
        trif = cf_sb[:, CF_TRI:CF_TRI + 128]
        esumf = cf_sb[:, CF_ESUM:CF_ESUM + 4]
        onescolf = cf_sb[:, CF_ONESC:CF_ONESC + 1]

        # ---- persistent per-token state (f16 where exact: masks/keep/kf
        # are small integers, so 2x DVE throughput is free) ---------------
        masks_sb = sb.tile([P, NTILE * E], F16, tag="masks")
        gate_sb = sb.tile([P, NTILE], F32, tag="gate")
        gate16_sb = sb.tile([P, NTILE], F16, tag="gate16")
        s_sb = sb.tile([P, NTILE], F32, tag="s")
        kf_sb = sb.tile([P, NTILE], F16, tag="kf")
        bc_sb = sb.tile([4, NTILE * E], F32, tag="bc")

        # ================= PHASE A: router + masks + counts ===============
        logits_sb = sb.tile([P, NTILE * E], F32, tag="logits")
        big_sb = sb.tile([P, NTILE * 2 * E], F32, tag="big")
        with ExitStack() as pa:
            plg = pa.enter_context(tc.tile_pool(name="plg", bufs=1, space="PSUM"))
            ptp = pa.enter_context(tc.tile_pool(name="ptp", bufs=2, space="PSUM"))
            psml = pa.enter_context(tc.tile_pool(name="psml", bufs=1, space="PSUM"))

            # router: 4 groups x 16 fp16 matmuls, all accumulated per group
            lgss = []
            for g in range(4):
                lgT = plg.tile([2 * E, 512], F32, space="PSUM", tag=f"lgT{g}")
                for c in range(NCH):
                    nc.tensor.matmul(
                        lgT[:], lhsT=wg_sb[:, c * 2 * E:(c + 1) * 2 * E],
                        rhs=xh3[:, c, g * 512:(g + 1) * 512],
                        start=(c == 0), stop=False)
                    nc.tensor.matmul(
                        lgT[:], lhsT=wg_sb[:, c * 2 * E:(c + 1) * 2 * E],
                        rhs=xl3[:, c, g * 512:(g + 1) * 512],
                        start=False, stop=(c == NCH - 1))
                lgs = st.tile([2 * E, 512], F32, tag="lgs")
                nc.vector.tensor_copy(lgs[:], lgT[:])
                lgss.append(lgs)

            # transpose all 16 tiles back: [16, 128] -> [128, 16], batched 4
            for b in range(4):
                tp4 = ptp.tile([P, 4 * 2 * E], F32, space="PSUM", tag="tp4")
                for j in range(4):
                    i = 4 * b + j
                    nc.tensor.transpose(
                        tp4[:, j * 2 * E:(j + 1) * 2 * E],
                        lgss[i // 4][:, (i % 4) * P:(i % 4 + 1) * P],
                        ident[:2 * E, :2 * E])
                nc.vector.tensor_copy(
                    big_sb[:, b * 4 * 2 * E:(b + 1) * 4 * 2 * E], tp4[:])

            # fold hi/lo halves: logits[:, (i,e)] = big[:, (i, e)] + big[:, (i, 8+e)]
            big3 = big_sb[:].rearrange("p (i e) -> p i e", e=2 * E)
            lg3 = logits_sb[:].rearrange("p (i e) -> p i e", e=E)
            nc.vector.tensor_tensor(lg3, big3[:, :, 0:E], big3[:, :, E:2 * E],
                                    op=ALU.add)

            # batched softmax / first-max mask over [128, 16, 8]
            m16 = st.tile([P, NTILE], F32, tag="m16")
            nc.vector.reduce_max(m16[:], lg3, axis=AX.X)
            m16b = m16[:].rearrange("p (i o) -> p i o", o=1).to_broadcast(
                [P, NTILE, E])
            d128 = st.tile([P, NTILE * E], F32, tag="d128")
            nc.vector.tensor_tensor(
                d128[:].rearrange("p (i e) -> p i e", e=E), lg3, m16b,
                op=ALU.subtract)
            e128 = st.tile([P, NTILE * E], F32, tag="e128")
            nc.scalar.activation(e128[:], d128[:], ACT.Exp)
            z16 = st.tile([P, NTILE], F32, tag="z16")
            nc.vector.reduce_sum(
                z16[:], e128[:].rearrange("p (i e) -> p i e", e=E), axis=AX.X)
            nc.vector.reciprocal(gate_sb[:], z16[:])
            nc.vector.tensor_copy(gate16_sb[:], gate_sb[:])
            mraw = st.tile([P, NTILE * E], F32, tag="mraw")
            nc.vector.tensor_tensor(
                mraw[:].rearrange("p (i e) -> p i e", e=E), lg3, m16b,
                op=ALU.is_equal)
            c1 = mraw
            for sh in (1, 2, 4):
                c2 = st.tile([P, NTILE * E], F32, tag=f"cc{sh}")
                c1v = c1[:].rearrange("p (i e) -> p i e", e=E)
                c2v = c2[:].rearrange("p (i e) -> p i e", e=E)
                nc.vector.tensor_copy(c2v[:, :, :sh], c1v[:, :, :sh])
                nc.vector.tensor_tensor(c2v[:, :, sh:], c1v[:, :, sh:],
                                        c1v[:, :, :E - sh], op=ALU.add)
                c1 = c2
            nc.vector.tensor_scalar(masks_sb[:], c1[:], 1.0, None,
                                    op0=ALU.is_equal)
            nc.vector.tensor_tensor(masks_sb[:], masks_sb[:], mraw[:],
                                    op=ALU.mult)

            # per-block expert counts, one matmul (f16: counts <= 32 exact)
            bcp = psml.tile([4, NTILE * E], F32, space="PSUM", tag="bcp")
            nc.tensor.matmul(bcp[:], lhsT=esumf, rhs=masks_sb[:],
                             start=True, stop=True)
            nc.vector.tensor_copy(bc_sb[:], bcp[:])

        # ================= AllGather of per-block counts ==================
        agin = dram.tile([NBLK, E], F32, tag="agin")
        agout = dram.tile([NCORES * NBLK, E], F32, tag="agout")
        nc.sync.dma_start(agin[:].rearrange("(i q) e -> q i e", q=4),
                          bc_sb[:].rearrange("p (i e) -> p i e", e=E))
        # own-counts readback runs before/during the collective
        bc64_sb = sb.tile([NBLK, E], F32, tag="bc64")
        nc.sync.dma_start(bc64_sb[:], agin[:])
        nc.gpsimd.collective_compute(
            "AllGather", ALU.bypass,
            replica_groups=[list(range(NCORES))],
            ins=[agin[:].opt()], outs=[agout[:].opt()])
        agt_sb = sb.tile([P, 4 * E], F32, tag="agt")
        nc.sync.dma_start(agt_sb[:].rearrange("p (c e) -> p c e", c=4),
                          agout[:].rearrange("(c p) e -> p c e", p=P))

        with ExitStack() as pb:
            psml = pb.enter_context(tc.tile_pool(name="psml2", bufs=2, space="PSUM"))
            ploc = pb.enter_context(tc.tile_pool(name="ploc", bufs=1, space="PSUM"))
            pcmp = pb.enter_context(tc.tile_pool(name="pcmp", bufs=1, space="PSUM"))

            # AG-independent matmuls first: they run inside the stall.
            loc4s = []
            for g in range(4):
                loc4 = ploc.tile([P, 32], F32, space="PSUM", tag=f"loc{g}")
                nc.tensor.matmul(loc4[:], lhsT=trif,
                                 rhs=masks_sb[:, 32 * g:32 * (g + 1)],
                                 start=True, stop=False)
                loc4s.append(loc4)
            ab = psml.tile([NBLK, E], F32, space="PSUM", tag="sm")
            nc.tensor.matmul(ab[:], lhsT=cc_sb[0:64, CC_H64:CC_H64 + NBLK],
                             rhs=bc64_sb[:], start=True, stop=False)
            nc.tensor.matmul(ab[:], lhsT=ones1[:, :NBLK], rhs=neg1,
                             start=False, stop=False)
            for c in range(4):
                nc.tensor.matmul(
                    ab[:],
                    lhsT=cc_sb[:, CC_MASKK + c * NBLK:CC_MASKK + (c + 1) * NBLK],
                    rhs=agt_sb[:, c * E:(c + 1) * E],
                    start=False, stop=(c == 3))
            ab_sb = sb.tile([NBLK, E], F32, tag="ab64")
            nc.vector.tensor_copy(ab_sb[:], ab[:])
            # repack [64, 8] (j, e) -> [4, 128] (q, (i, e)) on PE, no DRAM
            ab_exp = st.tile([NBLK, NTILE * E], F16, tag="abexp")
            ab_bc = ab_sb[:].rearrange("p (o e) -> p o e", o=1).to_broadcast(
                [NBLK, NTILE, E])
            nc.vector.tensor_tensor(
                ab_exp[:].rearrange("p (i e) -> p i e", e=E),
                dexp.rearrange("p (i e) -> p i e", e=E), ab_bc, op=ALU.mult)
            adp = psml.tile([4, NTILE * E], F32, space="PSUM", tag="sm")
            nc.tensor.matmul(adp[:], lhsT=q64, rhs=ab_exp[:],
                             start=True, stop=True)
            addbase_sb = sb.tile([4, NTILE * E], F16, tag="addbase")
            nc.vector.tensor_copy(addbase_sb[:], adp[:])

            # ============== PHASE B: keep / gate-scale / compaction =======
            keep_sb = sb.tile([P, NTILE * E], F16, tag="keep")
            for g in range(4):
                loc4 = loc4s[g]
                nc.tensor.matmul(loc4[:], lhsT=eexp16,
                                 rhs=addbase_sb[:, 32 * g:32 * (g + 1)],
                                 start=False, stop=True)
                nc.vector.tensor_scalar(keep_sb[:, 32 * g:32 * (g + 1)],
                                        loc4[:], float(C), None, op0=ALU.is_lt)
            nc.vector.tensor_tensor(keep_sb[:], keep_sb[:], masks_sb[:],
                                    op=ALU.mult)
            keep3 = keep_sb[:].rearrange("p (i e) -> p i e", e=E)
            with nc.allow_low_precision(reason="kf counts <= 8, exact in f16"):
                nc.vector.reduce_sum(kf_sb[:], keep3, axis=AX.X)
            g16b = gate16_sb[:].rearrange("p (i o) -> p i o",
                                          o=1).to_broadcast([P, NTILE, E])
            s128 = st.tile([P, NTILE * E], F16, tag="s128")
            nc.vector.tensor_tensor(
                s128[:].rearrange("p (i e) -> p i e", e=E), keep3, g16b,
                op=ALU.mult)
            nc.vector.reduce_sum(
                s_sb[:], s128[:].rearrange("p (i e) -> p i e", e=E), axis=AX.X)
            # per-tile kept counts, transposed: tksT[i] = sum_p kf[p, i]
            tksT = psml.tile([NTILE, 1], F32, space="PSUM", tag="sm")
            nc.tensor.matmul(tksT[:], lhsT=kf_sb[:], rhs=onescolf,
                             start=True, stop=True)
            tksT_sb = sb.tile([NTILE, 1], F32, tag="tksT")
            nc.vector.tensor_copy(tksT_sb[:], tksT[:])
            # exclusive prefix - 1 via matmul: posbT[i] = sum_{i'<i} tks[i'] - 1
            posbT = psml.tile([NTILE, 1], F32, space="PSUM", tag="sm")
            nc.tensor.matmul(posbT[:], lhsT=lt16, rhs=tksT_sb[:],
                             start=True, stop=False)
            nc.tensor.matmul(posbT[:], lhsT=ones1[:, :NTILE],
                             rhs=neg1[:, 0:1], start=False, stop=True)
            posbT_sb = sb.tile([NTILE, 1], F32, tag="posbT")
            nc.vector.tensor_copy(posbT_sb[:], posbT[:])
            diag16 = st.tile([16, 16], F32, tag="diag16")
            nc.vector.tensor_scalar(diag16[:], ident[:16, :16],
                                    posbT_sb[:, 0:1], None, op0=ALU.mult)

            # slot position per token: within-tile rank + tile base
            pos16 = psml.tile([P, NTILE], F32, space="PSUM", tag="sm")
            nc.tensor.matmul(pos16[:], lhsT=trif, rhs=kf_sb[:],
                             start=True, stop=False)
            nc.tensor.matmul(pos16[:], lhsT=ones16, rhs=diag16[:],
                             start=False, stop=True)
            notk = st.tile([P, NTILE], F32, tag="notk")
            nc.vector.tensor_scalar(notk[:], kf_sb[:], 0.5, 4096.0,
                                    op0=ALU.is_lt, op1=ALU.mult)
            poss = st.tile([P, NTILE], F32, tag="poss")
            nc.vector.tensor_tensor(poss[:], pos16[:], notk[:], op=ALU.add)

            # value pairs [token-idx ; gate-scale] per tile (f16: idx <= 2047
            # exact, gate rounds 2^-11 which is far inside tolerance)
            tsv_sb = sb.tile([P, 2 * NTILE], F16, tag="tsv")
            tsv3 = tsv_sb[:].rearrange("p (i j) -> p i j", j=2)
            nc.vector.tensor_copy(
                tsv3[:, :, 0:1], tidx.rearrange("p (i o) -> p i o", o=1))
            nc.vector.tensor_copy(
                tsv3[:, :, 1:2], s_sb[:].rearrange("p (i o) -> p i o", o=1))

            # one-hot M for all tiles (16 back-to-back vector ops), then the
            # 16 compaction matmuls back-to-back (f16: 1 cyc/row, no fp32
            # double-pump)
            M_all = sb.tile([P, NTILE * KMAX], F16, tag="Mall")
            for i in range(NTILE):
                nc.vector.tensor_scalar(M_all[:, i * KMAX:(i + 1) * KMAX],
                                        iota16f, poss[:, i:i + 1], None,
                                        op0=ALU.is_equal)
            cmpT = pcmp.tile([2, KMAX], F32, space="PSUM", tag="cmpT")
            for i in range(NTILE):
                nc.tensor.matmul(cmpT[:], lhsT=tsv_sb[:, 2 * i:2 * i + 2],
                                 rhs=M_all[:, i * KMAX:(i + 1) * KMAX],
                                 start=(i == 0), stop=(i == NTILE - 1))

            # extract: transpose [2,128] -> [128,2] on the PE
            cmpT_sb = sb.tile([2, KMAX], F32, tag="cmpTsb")
            nc.vector.tensor_copy(cmpT_sb[:], cmpT[:])
            gst = psml.tile([P, 2], F32, space="PSUM", tag="sm")
            nc.tensor.transpose(gst[:], cmpT_sb[:], ident[:2, :2])
            gs_sb = sb.tile([P, 2], F32, tag="gs")   # col 0 = idx, 1 = s
            nc.vector.tensor_copy(gs_sb[:], gst[:])
            scmp = gs_sb[:, 1:2]
            gidx = sb.tile([P, 1], I32, tag="gidx")
            nc.vector.tensor_copy(gidx[:], gs_sb[:, 0:1])
            # pad slots have idx 0 and s 0 -> route them to the trash row
            padf = st.tile([P, 1], F32, tag="padf")
            nc.vector.tensor_scalar(padf[:], scmp, 0.0, float(TRASH),
                                    op0=ALU.is_equal, op1=ALU.mult)
            gsf = st.tile([P, 1], F32, tag="gsf")
            nc.vector.tensor_tensor(gsf[:], gs_sb[:, 0:1], padf[:],
                                    op=ALU.add)
            sidx = sb.tile([P, 1], I32, tag="sidx")
            nc.vector.tensor_copy(sidx[:], gsf[:])

        # ============== PHASE C: gather, expert matmul, scatter ===========
        with ExitStack() as pc:
            ptp = pc.enter_context(tc.tile_pool(name="ptp2", bufs=2,
                                                space="PSUM"))
            ppo = pc.enter_context(tc.tile_pool(name="ppo", bufs=2,
                                                space="PSUM"))
            xg = st.tile([P, H], F16, tag="xg")
            nc.gpsimd.indirect_dma_start(
                out=xg[:], out_offset=None, in_=x[:, :],
                in_offset=bass.IndirectOffsetOnAxis(ap=gidx[:, :1], axis=0))
            xgT = st.tile([P, H], F16, tag="xgT")
            for g2 in range(2):
                tp = ptp.tile([P, 512], F16, space="PSUM", tag="tp2")
                for c4 in range(4):
                    c = g2 * 4 + c4
                    nc.tensor.transpose(tp[:, c4 * P:(c4 + 1) * P],
                                        xg[:, c * P:(c + 1) * P], identf)
                nc.vector.tensor_copy(xgT[:, g2 * 512:(g2 + 1) * 512], tp[:])

            for n, outn in enumerate((out0, out1)):
                po = ppo.tile([P, 512], F32, space="PSUM", tag="po")
                for c in range(NCH):
                    nc.tensor.matmul(
                        po[:], lhsT=xgT[:, c * P:(c + 1) * P],
                        rhs=we_sb[:, c * H + n * 512: c * H + (n + 1) * 512],
                        start=(c == 0), stop=(be_sb is None and c == NCH - 1))
                if be_sb is not None:
                    # trif row 0 is all-ones
                    nc.tensor.matmul(po[:], lhsT=trif[0:1, :],
                                     rhs=be_sb[0:1, n * 512:(n + 1) * 512],
                                     start=False, stop=True)
                # gate scale folded into the PSUM->SBUF copy; scatter each
                # column half as soon as it is ready
                outsb = st.tile([P, 512], F32, tag=f"outsb{n}")
                nc.vector.tensor_scalar(outsb[:], po[:], scmp[:, :1], None,
                                        op0=ALU.mult)
                nc.gpsimd.indirect_dma_start(
                    out=outn[:, :],
                    out_offset=bass.IndirectOffsetOnAxis(ap=sidx[:, :1],
                                                         axis=0),
                    in_=outsb[:], in_offset=None)


# ---------------------------------------------------------------------------
# host side
# ---------------------------------------------------------------------------

def make_consts():
    ident = np.eye(P, dtype=np.float32)
    blk_of = np.arange(P) // BLK
    esum = (blk_of[:, None] == np.arange(4)[None, :]).astype(np.float32)
    ch = np.concatenate([ident, esum], axis=1)           # [128, 132]

    cold = np.zeros((P, CC_W), np.float32)
    cold[:, CC_TRI:CC_TRI + 128] = np.triu(np.ones((P, P), np.float32))
    cold[:, CC_IOTA:CC_IOTA + 128] = np.tile(
        np.arange(KMAX, dtype=np.float32)[None, :], (P, 1))
    cold[:, CC_TIDX:CC_TIDX + NTILE] = (
        np.arange(NTILE, dtype=np.float32)[None, :] * P
        + np.arange(P, dtype=np.float32)[:, None])
    cold[0:4, CC_MISC:CC_MISC + 128] = esum.T
    cold[0:16, CC_ONES1:CC_ONES1 + 128] = 1.0
    cold[:, CC_ONESC] = 1.0
    cold[0:1, CC_NEG1:CC_NEG1 + E] = -1.0
    j = np.arange(NBLK)
    cold[0:64, CC_H64:CC_H64 + NBLK] = (
        j[:, None] < 4 * (j[None, :] // 4)).astype(np.float32)
    k16 = np.arange(NTILE)
    cold[0:16, CC_LT16:CC_LT16 + 16] = (
        k16[:, None] < k16[None, :]).astype(np.float32)
    # Dexp[j, 8i+e] = (j//4 == i)
    i16 = np.arange(NTILE)
    cold[0:64, CC_DEXP:CC_DEXP + 128] = np.repeat(
        (j[:, None] // 4 == i16[None, :]).astype(np.float32), E, axis=1)

    cf = np.zeros((P, CF_W), np.float16)
    cf[0:4, CF_EEXP:CF_EEXP + 128] = esum.T.astype(np.float16)
    # Q64[j, q] = (j % 4 == q)
    cf[0:64, CF_Q64:CF_Q64 + 4] = (
        j[:, None] % 4 == np.arange(4)[None, :]).astype(np.float16)
    cf[:, CF_IOTA:CF_IOTA + 128] = np.tile(
        np.arange(KMAX, dtype=np.float16)[None, :], (P, 1))
    cf[:, CF_IDENT:CF_IDENT + 128] = ident.astype(np.float16)
    cf[:, CF_TRI:CF_TRI + 128] = np.triu(np.ones((P, P), np.float16))
    cf[:, CF_ESUM:CF_ESUM + 4] = esum.astype(np.float16)
    cf[:, CF_ONESC] = 1.0
    return ch, cold, cf


def make_maskk(k):
    # rows (r*64 + jp) = foreign core r's local block jp (global block 8*jp+r)
    # cols j = my local block (global 8*j + k)
    r = np.arange(NCORES)[:, None, None]
    jp = np.arange(NBLK)[None, :, None]
    jm = np.arange(NBLK)[None, None, :]
    m = (r != k) & (8 * jp + r < 8 * jm + k)
    return m.astype(np.float32).reshape(NCORES * NBLK, NBLK)


def make_in_maps(x, w_gate, w_expert, b_expert):
    xf = np.ascontiguousarray(np.asarray(x, np.float32).reshape(-1, H))
    xb = xf.reshape(-1, BLK, H)          # (512, 32, H)
    ch, cold, cf = make_consts()
    wgf = np.asarray(w_gate, np.float32)
    wg_h = wgf.astype(np.float16)
    wg_l = (wgf - wg_h.astype(np.float32)).astype(np.float16)
    wgcat = np.ascontiguousarray(np.concatenate([wg_h, wg_l], axis=1))
    wef = np.ascontiguousarray(np.asarray(w_expert, np.float32).astype(
        np.float16))
    bef = np.asarray(b_expert, np.float32).reshape(1, H)
    has_bias = bool(np.any(bef))
    in_maps = []
    for k in range(NCORES):
        shard = np.ascontiguousarray(xb[k::NCORES].reshape(T_LOC, H))
        sh_h = shard.astype(np.float16)
        sh_l = (shard - sh_h.astype(np.float32)).astype(np.float16)
        ccold = cold.copy()
        # maskk [512, 64] -> [128, 4, 64]
        ccold[:, CC_MASKK:CC_MASKK + 4 * NBLK] = make_maskk(k).reshape(
            4, P, NBLK).transpose(1, 0, 2).reshape(P, 4 * NBLK)
        m = {"x16": sh_h,
             "xth": np.ascontiguousarray(sh_h.T),
             "xtl": np.ascontiguousarray(sh_l.T),
             "wgcat": wgcat, "w_expert": wef,
             "ch": ch, "cc": ccold, "cf": cf}
        if has_bias:
            m["b_expert"] = np.ascontiguousarray(bef.astype(np.float16))
        in_maps.append(m)
    return in_maps


def assemble_out(results, batch_shape):
    T = NCORES * T_LOC
    outf = np.empty((T // BLK, BLK, H), np.float32)
    for k in range(NCORES):
        full = np.concatenate([results[k]["out0"][:T_LOC],
                               results[k]["out1"][:T_LOC]], axis=1)
        outf[k::NCORES] = full.reshape(-1, BLK, H)
    return outf.reshape(batch_shape)


_NC = None
_NC_BIAS = None
LAST_EXEC_NS = None


def _maybe_register_ntff_hook():
    """Best-effort registration of the axon NTFF profiling hook (used only
    when BASS_TRACE is set); harmless if unavailable."""
    try:
        import antenv
        from trn_agent_boot.trn_boot import _ntff_profile_via_ctypes
        if "antenv.axon_hooks" in sys.modules:
            return
        hook = _ntff_profile_via_ctypes("/opt/axon/libaxon_pjrt.so")
        mod = types.ModuleType("antenv.axon_hooks")
        mod.get_axon_ntff_profile_hook = lambda: hook
        mod.set_axon_ntff_profile_hook = lambda h: None
        antenv.axon_hooks = mod
        sys.modules["antenv.axon_hooks"] = mod
        bass_utils.upload_artifacts = lambda tmpdir: f"file://{tmpdir}"
    except Exception:
        pass


def kernel(x, w_gate, w_expert, b_expert):
    global _NC, _NC_BIAS, LAST_EXEC_NS
    if os.environ.get("BASS_TRACE"):
        _maybe_register_ntff_hook()
    in_maps = make_in_maps(x, w_gate, w_expert, b_expert)
    has_bias = "b_expert" in in_maps[0]
    if has_bias:
        if _NC_BIAS is None:
            _NC_BIAS = build(has_bias=True)
        prog = _NC_BIAS
    else:
        if _NC is None:
            _NC = build(has_bias=False)
        prog = _NC
    # The fleet occasionally throws a transient NRT_EXEC_UNIT_UNRECOVERABLE
    # on execute (observed ~10% of invocations).  A plain in-process retry
    # does NOT recover (the wedged device state is cached in the PJRT
    # backend), so tear the jax backend down between attempts.
    last_exc = None
    for attempt in range(4):
        try:
            res = bass_utils.run_bass_kernel_spmd(
                prog, in_maps, core_ids=list(range(NCORES)))
            break
        except Exception as exc:
            last_exc = exc
            import time as _time
            _time.sleep(3.0)
            try:
                import jax
                jax.clear_caches()
                jax.extend.backend.clear_backends()
            except Exception:
                try:
                    import jax
                    jax.clear_backends()
                except Exception:
                    pass
            _time.sleep(2.0)
    else:
        raise last_exc
    LAST_EXEC_NS = res.exec_time_ns
    return assemble_out(res.results, np.asarray(x).shape)
